# revision 18
# baseline (speedup 1.0000x reference)
"""CFAR box-filter kernel (31x31 / 11x11 box sums + ratio) for Trainium2.

Data-parallel over batch: 32 images -> 8 NeuronCores, 4 images each.
Per 128-row chunk:
  - two fused sliding-window scans on DVE compute the horizontal 11- and
    31-wide box sums directly (fp32 carry, fp16 output) -- no prefix
    tensor, no subtract passes,
  - vertical box sums as banded fp16 matmuls (weights carry the 1/121
    and +-1/840 output scales; halo rows copied as one 15-row slab from
    each neighbor chunk's h tile),
  - front lands in PSUM (ACT copies it out as fp16), back lands in PSUM
    (DVE fast reciprocal, DVE multiply -> ratio, fp16 out).
GPSIMD is left idle: its SBUF port is an exclusive lock shared with DVE,
so concurrent GPSIMD elementwise work serializes against DVE anyway.
"""

import os
import sys

import numpy as np

for _p in ("/opt/trn_rl_repo", "/root/.axon_site/_ro/trn_rl_repo"):
    if os.path.isdir(_p) and _p not in sys.path:
        sys.path.insert(0, _p)
        break

import concourse.bass as bass
import concourse.tile as tile
from concourse import bacc
from concourse import mybir
from concourse._compat import with_exitstack
from concourse.bass_utils import run_bass_kernel_spmd

B, H, W = 32, 1024, 1024
NCORES = 8
BPC = B // NCORES            # images per core
CHUNKS = H // 128            # row chunks per image
PADL, PADR = 32, 16
SCANW = PADL + W + PADR      # 1072
F16 = mybir.dt.float16
F32 = mybir.dt.float32

GUARD, BG = 5, 10
R_IN = GUARD                 # 11x11 radius
R_OUT = GUARD + BG           # 31x31 radius
AREA_FRONT = float((2 * R_IN + 1) ** 2)                        # 121
AREA_BACK = float((2 * R_OUT + 1) ** 2 - (2 * R_IN + 1) ** 2)  # 840

# Fused scans: state[t] = sum(xs[t+1 : t+k+1]); valid because xs cols
# [0, k) are zero (PADL = 32 >= 31).  Pixel w's centered k-window ends at
# tile col w + PADL + r, so h_k[w] = out[w + PADL + r - k].
L11 = W + PADL + R_IN - 11   # 1050 scan length for k=11
L31 = W + PADL + R_OUT - 31  # 1040 scan length for k=31
O11 = PADL + R_IN - 11       # 26  h11[w] = out11[w + O11]
O31 = PADL + R_OUT - 31      # 16  h31[w] = out31[w + O31]
HOFF31 = L11                 # h31 columns start here in the merged h tile
HW_ = L11 + L31              # 2090 merged h tile width

SCAN31_ON_GPS = os.environ.get("CFAR_SCAN31_GPS", "0") == "1"
# Even-lag variant: 12/32-wide window scans (4B-aligned fp16 operands, DVE
# 2x-mode eligible) + one 2x fp16 subtract each to recover the 11/31 windows.
EVENSCAN = os.environ.get("CFAR_EVENSCAN", "0") == "1"
# Fused ratio = front * recip(back) in one custom-DVE pass (seed + 1 Newton
# step, ~0.17% max rel err) instead of reciprocal_approx_fast + tensor_mul.
FUSED_RATIO = os.environ.get("CFAR_FUSED_RATIO", "0") == "1"
RECIP_C0, RECIP_C1 = -0.23549792, 2.0017324


def _register_ratio_op():
    """Register out = Src1 * recip1NR(Src0) as a custom DVE op (in-process)."""
    from concourse import dve_ops as dops
    from concourse.dve_spec import AluOp, Bin, C0, C1, Spec, Src0, Src1
    from concourse.dve_spec import _has_src1, lower
    from concourse.dve_uop import DveOpSpec

    name = "RATIO_RECIP1NR_ANT"
    for op in dops.OPS:
        if op.name == name:
            return op
    _not = Bin(AluOp.BITWISE_NOT, Src0, Src0)
    _y0 = _not * C0

    def _ref(in0, in1, c0, c1, c2):
        not_x = (~in0.view(np.int32)).view(np.float32)
        y0 = not_x * np.float32(c0)
        y1 = y0 * (np.float32(c1) - in0 * y0)
        return y1 * in1

    spec = Spec(body=(_y0 * (C1 - Src0 * _y0)) * Src1, reference=_ref)
    row = dops._CUSTOM_DVE_ROW_BASE + len(dops.OPS)
    dops._SUB_OPCODE_FOR_NAME[name] = row
    shas = {}
    for ver in ("v3", "v4"):
        s = DveOpSpec(
            name=name, opcode=row, uops=lower(spec, ver=ver),
            rd1_en=_has_src1(spec),
        )
        shas[ver] = s.sha(ver)
    op = dops.DveOp(name, spec, subdim=False, uops_sha=shas)
    dops.OPS.append(op)
    dops.CUSTOM_DVE_SPECS[name] = spec
    return op


RATIO_OP = _register_ratio_op() if FUSED_RATIO else None


def _weights() -> dict[str, np.ndarray]:
    # Halo slab row -> virtual image row (relative to the chunk's rows
    # 0..127).  Inner chunks: slab rows 0..14 = prev chunk's last 15 rows
    # (virtual -15..-1), rows 15..29 = next chunk's first 15 (128..142).
    # Edge chunks use a 15-row slab at partition offset 0 (matmul operands
    # must start at partition 0/32/64), so P/N variants get their own
    # 15-row weights; rows outside a filter's reach are all-zero columns.
    k = np.arange(128)[:, None]
    m = np.arange(128)[None, :]
    g30 = np.arange(2 * R_OUT)[:, None]
    g30 = np.where(g30 < R_OUT, g30 - R_OUT, 128 + (g30 - R_OUT))
    gP = np.arange(R_OUT)[:, None] - R_OUT          # prev-only slab
    gN = 128 + np.arange(R_OUT)[:, None]            # next-only slab

    def band(gg, radius, scale):
        return ((np.abs(gg - m) <= radius) * scale).astype(np.float16)

    w = {
        "wf_blk": band(k, R_IN, 1.0 / AREA_FRONT),
        "wb31_blk": band(k, R_OUT, 1.0 / AREA_BACK),
        "wn11_blk": band(k, R_IN, -1.0 / AREA_BACK),
    }
    for sfx, gg in (("", g30), ("P", gP), ("N", gN)):
        w["wf_halo" + sfx] = band(gg, R_IN, 1.0 / AREA_FRONT)
        w["wb31_halo" + sfx] = band(gg, R_OUT, 1.0 / AREA_BACK)
        w["wn11_halo" + sfx] = band(gg, R_IN, -1.0 / AREA_BACK)
    return w


@with_exitstack
def _cfar_tile_kernel(ctx, tc, x_d, o_d, w_d, n_img, reps=1):
    nc = tc.nc
    ADD = mybir.AluOpType.add
    SUB = mybir.AluOpType.subtract

    const = ctx.enter_context(tc.tile_pool(name="const", bufs=1))
    wt = {}
    for name, dram_ap in w_d.items():
        t = const.tile(list(dram_ap.shape), F16, tag=name)
        nc.sync.dma_start(t[:], dram_ap)
        wt[name] = t

    xp = ctx.enter_context(tc.tile_pool(name="xp", bufs=3))
    hp = ctx.enter_context(tc.tile_pool(name="hp", bufs=4))
    scp = ctx.enter_context(tc.tile_pool(name="scp", bufs=2)) if EVENSCAN else None
    gp = ctx.enter_context(tc.tile_pool(name="gp", bufs=3))
    pp = ctx.enter_context(tc.tile_pool(name="pp", bufs=2, space="PSUM"))
    rp = ctx.enter_context(tc.tile_pool(name="rp", bufs=3))
    obp = ctx.enter_context(tc.tile_pool(name="obp", bufs=3))

    # PE clock warm-up: ~4.5us of back-to-back matmuls into a scratch PSUM
    # tile (recycled by the pool for the first real chunk) so the HAM clock
    # gate reaches 8/8 before the first chunk's matmuls issue.
    warm = pp.tile([128, W], F32, tag="front")
    for _ in range(80):
        nc.tensor.matmul(
            warm[:, 0:128], wt["wb31_blk"][:], wt["wb31_blk"][:],
            start=True, stop=True,
        )

    scan31_eng = nc.gpsimd if SCAN31_ON_GPS else nc.vector

    def one_pass():
      for img in range(n_img):
        hs: dict[int, object] = {}

        def produce(t):
            xt = xp.tile([128, SCANW], F16, tag="xt")
            nc.sync.dma_start(xt[:], x_d[img, 128 * t : 128 * (t + 1), :])
            ht = hp.tile([128, HW_], F16, tag="ht")
            if EVENSCAN:
                # even-lag scans (4B-aligned fp16 -> 2x eligible):
                #   s12[t] = sum(xs[t+1 : t+13]),  s32[t] = sum(xs[t+1 : t+33])
                # then h11[w] = s12[w+26] - xs[w+38]  (11-window at w)
                #      h31[w] = s32[w+16] - xs[w+48]  (31-window at w)
                sc = scp.tile([128, HW_], F16, tag="sc")
                nc.vector.tensor_tensor_scan(
                    sc[:, 0:L11], xt[:, 12 : 12 + L11], xt[:, 0:L11], 0.0, ADD, SUB
                )
                nc.vector.tensor_tensor_scan(
                    sc[:, HOFF31 : HOFF31 + L31],
                    xt[:, 32 : 32 + L31],
                    xt[:, 0:L31],
                    0.0,
                    ADD,
                    SUB,
                )
                nc.vector.tensor_sub(
                    ht[:, O11 : O11 + W], sc[:, 26 : 26 + W], xt[:, 38 : 38 + W]
                )
                nc.vector.tensor_sub(
                    ht[:, HOFF31 + O31 : HOFF31 + O31 + W],
                    sc[:, HOFF31 + 16 : HOFF31 + 16 + W],
                    xt[:, 48 : 48 + W],
                )
            else:
                # h11: state = (xs[t+11] + state) - xs[t]
                nc.vector.tensor_tensor_scan(
                    ht[:, 0:L11], xt[:, 11 : 11 + L11], xt[:, 0:L11], 0.0, ADD, SUB
                )
                # h31: state = (xs[t+31] + state) - xs[t]
                scan31_eng.tensor_tensor_scan(
                    ht[:, HOFF31 : HOFF31 + L31],
                    xt[:, 31 : 31 + L31],
                    xt[:, 0:L31],
                    0.0,
                    ADD,
                    SUB,
                )
            hs[t] = ht

        def consume(t):
            # halo slab: inner chunks rows 0..14 = prev ht[113:128],
            # 15..29 = next ht[0:15]; edge chunks use a 15-row slab.
            g = gp.tile([2 * R_OUT, HW_], F16, tag="g")
            if t == 0:
                nc.sync.dma_start(g[0:R_OUT, :], hs[t + 1][0:R_OUT, :])
                sfx, gsl = "N", slice(0, R_OUT)
            elif t == CHUNKS - 1:
                nc.sync.dma_start(g[0:R_OUT, :], hs[t - 1][128 - R_OUT : 128, :])
                sfx, gsl = "P", slice(0, R_OUT)
            else:
                nc.sync.dma_start(g[0:R_OUT, :], hs[t - 1][128 - R_OUT : 128, :])
                nc.sync.dma_start(g[R_OUT : 2 * R_OUT, :], hs[t + 1][0:R_OUT, :])
                sfx, gsl = "", slice(0, 2 * R_OUT)

            ht = hs[t]
            psf = pp.tile([128, W], F32, tag="front")
            psb = pp.tile([128, W], F32, tag="back")
            MM = nc.tensor.matmul
            # grouped by weight so consecutive matmuls share one LDWEIGHTS
            for wname, dst, rhs_of, st, sp in (
                ("wf_blk", psf, "h11", True, False),
                ("wf_halo" + sfx, psf, "g11", False, True),
                ("wb31_blk", psb, "h31", True, False),
                ("wb31_halo" + sfx, psb, "g31", False, False),
                ("wn11_blk", psb, "h11", False, False),
                ("wn11_halo" + sfx, psb, "g11", False, True),
            ):
                for h0 in (0, 512):
                    s = slice(h0, h0 + 512)
                    c11 = slice(O11 + h0, O11 + h0 + 512)
                    c31 = slice(HOFF31 + O31 + h0, HOFF31 + O31 + h0 + 512)
                    rhs = {
                        "h11": ht[:, c11],
                        "h31": ht[:, c31],
                        "g11": g[gsl, c11],
                        "g31": g[gsl, c31],
                    }[rhs_of]
                    MM(dst[:, s], wt[wname][:], rhs, start=st, stop=sp)

            o1 = obp.tile([128, W], F16, tag="o1")
            nc.scalar.copy(o1[:], psf[:])
            o0 = obp.tile([128, W], F16, tag="o0")
            if FUSED_RATIO:
                nc.vector._custom_dve(
                    RATIO_OP, out=o0[:], in0=psb[:], in1=o1[:],
                    s0=RECIP_C0, s1=RECIP_C1,
                )
            else:
                r = rp.tile([128, W], F32, tag="r")
                nc.vector.reciprocal_approx_fast(out=r[:], in_=psb[:])
                nc.vector.tensor_mul(o0[:], psf[:], r[:])
            rows = slice(128 * t, 128 * (t + 1))
            nc.scalar.dma_start(o_d[img, rows, :], o0[:])
            nc.scalar.dma_start(o_d[n_img + img, rows, :], o1[:])

        produce(0)
        produce(1)
        consume(0)
        for t in range(2, CHUNKS):
            produce(t)
            consume(t - 1)
        consume(CHUNKS - 1)

    if reps == 1:
        one_pass()
    else:
        with tc.For_i(0, reps, 1):
            one_pass()


def build(n_img: int = BPC, reps: int = 1):
    nc = bacc.Bacc("TRN2", target_bir_lowering=False, debug=False)
    x_d = nc.dram_tensor("x", [n_img, H, SCANW], F16, kind="ExternalInput").ap()
    o_d = nc.dram_tensor("out", [2 * n_img, H, W], F16, kind="ExternalOutput").ap()
    wts = _weights()
    w_d = {
        k: nc.dram_tensor(k, list(v.shape), F16, kind="ExternalInput").ap()
        for k, v in wts.items()
    }
    with tile.TileContext(nc) as tc:
        _cfar_tile_kernel(tc, x_d, o_d, w_d, n_img, reps)
    nc.compile()
    return nc, wts


_CACHE: dict = {}


def make_in_maps(x: np.ndarray, wts: dict) -> list[dict]:
    xs = np.zeros((B, H, SCANW), dtype=np.float16)
    xs[:, :, PADL : PADL + W] = x.reshape(B, H, W)
    in_maps = []
    for i in range(NCORES):
        m = {"x": np.ascontiguousarray(xs[BPC * i : BPC * (i + 1)])}
        m.update(wts)
        in_maps.append(m)
    return in_maps


def kernel(x: np.ndarray) -> np.ndarray:
    x = np.ascontiguousarray(np.asarray(x, dtype=np.float32))
    assert x.shape == (B, 1, H, W), x.shape
    if "nc" not in _CACHE:
        _CACHE["nc"], _CACHE["wts"] = build(BPC)
    nc, wts = _CACHE["nc"], _CACHE["wts"]
    in_maps = make_in_maps(x, wts)
    res = run_bass_kernel_spmd(nc, in_maps, list(range(NCORES))).results
    out = np.empty((2 * B, 1, H, W), dtype=np.float32)
    for i in range(NCORES):
        o = res[i]["out"].astype(np.float32)
        out[BPC * i : BPC * (i + 1), 0] = o[:BPC]
        out[B + BPC * i : B + BPC * (i + 1), 0] = o[BPC:]
    return out


# revision 21
# speedup vs baseline: 1.3118x; 1.3118x over previous
"""CFAR box-filter kernel (31x31 / 11x11 box sums + ratio) for Trainium2.

Data-parallel over batch: 32 images -> 8 NeuronCores, 4 images each.
Per 128-row chunk:
  - two fused sliding-window scans on DVE compute the horizontal 11- and
    31-wide box sums directly (fp32 carry, fp16 output) -- no prefix
    tensor, no subtract passes,
  - vertical box sums as banded fp16 matmuls (weights carry the 1/121
    and +-1/840 output scales; halo rows copied as one 15-row slab from
    each neighbor chunk's h tile),
  - front lands in PSUM (ACT copies it out as fp16), back lands in PSUM
    (DVE fast reciprocal, DVE multiply -> ratio, fp16 out).
GPSIMD is left idle: its SBUF port is an exclusive lock shared with DVE,
so concurrent GPSIMD elementwise work serializes against DVE anyway.
"""

import os
import sys

import numpy as np

for _p in ("/opt/trn_rl_repo", "/root/.axon_site/_ro/trn_rl_repo"):
    if os.path.isdir(_p) and _p not in sys.path:
        sys.path.insert(0, _p)
        break

import concourse.bass as bass
import concourse.tile as tile
from concourse import bacc
from concourse import mybir
from concourse._compat import with_exitstack
from concourse.bass_utils import run_bass_kernel_spmd

B, H, W = 32, 1024, 1024
NCORES = 8
BPC = B // NCORES            # images per core
CHUNKS = H // 128            # row chunks per image
PADL, PADR = 32, 16
SCANW = PADL + W + PADR      # 1072
F16 = mybir.dt.float16
F32 = mybir.dt.float32

GUARD, BG = 5, 10
R_IN = GUARD                 # 11x11 radius
R_OUT = GUARD + BG           # 31x31 radius
AREA_FRONT = float((2 * R_IN + 1) ** 2)                        # 121
AREA_BACK = float((2 * R_OUT + 1) ** 2 - (2 * R_IN + 1) ** 2)  # 840

# Fused scans: state[t] = sum(xs[t+1 : t+k+1]); valid because xs cols
# [0, k) are zero (PADL = 32 >= 31).  Pixel w's centered k-window ends at
# tile col w + PADL + r, so h_k[w] = out[w + PADL + r - k].
L11 = W + PADL + R_IN - 11   # 1050 scan length for k=11
L31 = W + PADL + R_OUT - 31  # 1040 scan length for k=31
O11 = PADL + R_IN - 11       # 26  h11[w] = out11[w + O11]
O31 = PADL + R_OUT - 31      # 16  h31[w] = out31[w + O31]
HOFF31 = L11                 # h31 columns start here in the merged h tile
HW_ = L11 + L31              # 2090 merged h tile width

SCAN31_ON_GPS = os.environ.get("CFAR_SCAN31_GPS", "0") == "1"
# Even-lag variant: 12/32-wide window scans (4B-aligned fp16 operands, DVE
# 2x-mode eligible) + one 2x fp16 subtract each to recover the 11/31 windows.
EVENSCAN = os.environ.get("CFAR_EVENSCAN", "0") == "1"
# Fused ratio = front * recip(back) in one custom-DVE pass, where
# back = in0 - c*in1 is folded in too (in0 = 31-box PSUM, in1 = front fp16).
# Reciprocal is bitwise-NOT seed + 1 Newton step: ~0.17% max rel err.
FUSED_RATIO = os.environ.get("CFAR_FUSED_RATIO", "1") == "1"
RECIP_C0, RECIP_C1 = -0.23549792, 2.0017324
NRM_C = AREA_FRONT / AREA_BACK  # back = T/840 - c*front


def _register_ratio_op():
    """Register out = Src1 * recip1NR(Src0 - C2*Src1) as a custom DVE op."""
    from concourse import dve_ops as dops
    from concourse.dve_spec import AluOp, Bin, C0, C1, C2, Spec, Src0, Src1
    from concourse.dve_spec import _has_src1, lower
    from concourse.dve_uop import DveOpSpec

    name = "RATIO_RECIP1NR_ANT"
    for op in dops.OPS:
        if op.name == name:
            return op
    _x = Src0 - Src1 * C2
    _not = Bin(AluOp.BITWISE_NOT, _x, _x)
    _y0 = _not * C0

    def _ref(in0, in1, c0, c1, c2):
        x = (in0 - in1.astype(np.float32) * np.float32(c2)).astype(np.float32)
        not_x = (~x.view(np.int32)).view(np.float32)
        y0 = not_x * np.float32(c0)
        y1 = y0 * (np.float32(c1) - x * y0)
        return y1 * in1

    spec = Spec(body=(_y0 * (C1 - _x * _y0)) * Src1, reference=_ref)
    row = dops._CUSTOM_DVE_ROW_BASE + len(dops.OPS)
    dops._SUB_OPCODE_FOR_NAME[name] = row
    shas = {}
    for ver in ("v3", "v4"):
        s = DveOpSpec(
            name=name, opcode=row, uops=lower(spec, ver=ver),
            rd1_en=_has_src1(spec),
        )
        shas[ver] = s.sha(ver)
    op = dops.DveOp(name, spec, subdim=False, uops_sha=shas)
    dops.OPS.append(op)
    dops.CUSTOM_DVE_SPECS[name] = spec
    return op


RATIO_OP = _register_ratio_op() if FUSED_RATIO else None


def _weights() -> dict[str, np.ndarray]:
    # Halo slab row -> virtual image row (relative to the chunk's rows
    # 0..127).  Inner chunks: slab rows 0..14 = prev chunk's last 15 rows
    # (virtual -15..-1), rows 15..29 = next chunk's first 15 (128..142).
    # Edge chunks use a 15-row slab at partition offset 0 (matmul operands
    # must start at partition 0/32/64), so P/N variants get their own
    # 15-row weights; rows outside a filter's reach are all-zero columns.
    k = np.arange(128)[:, None]
    m = np.arange(128)[None, :]
    g30 = np.arange(2 * R_OUT)[:, None]
    g30 = np.where(g30 < R_OUT, g30 - R_OUT, 128 + (g30 - R_OUT))
    gP = np.arange(R_OUT)[:, None] - R_OUT          # prev-only slab
    gN = 128 + np.arange(R_OUT)[:, None]            # next-only slab

    def band(gg, radius, scale):
        return ((np.abs(gg - m) <= radius) * scale).astype(np.float16)

    w = {
        "wf_blk": band(k, R_IN, 1.0 / AREA_FRONT),
        "wb31_blk": band(k, R_OUT, 1.0 / AREA_BACK),
        "wn11_blk": band(k, R_IN, -1.0 / AREA_BACK),
    }
    for sfx, gg in (("", g30), ("P", gP), ("N", gN)):
        w["wf_halo" + sfx] = band(gg, R_IN, 1.0 / AREA_FRONT)
        w["wb31_halo" + sfx] = band(gg, R_OUT, 1.0 / AREA_BACK)
        w["wn11_halo" + sfx] = band(gg, R_IN, -1.0 / AREA_BACK)
    return w


@with_exitstack
def _cfar_tile_kernel(ctx, tc, x_d, o_d, w_d, n_img, reps=1):
    nc = tc.nc
    ADD = mybir.AluOpType.add
    SUB = mybir.AluOpType.subtract

    const = ctx.enter_context(tc.tile_pool(name="const", bufs=1))
    wt = {}
    for name, dram_ap in w_d.items():
        t = const.tile(list(dram_ap.shape), F16, tag=name)
        nc.sync.dma_start(t[:], dram_ap)
        wt[name] = t

    xp = ctx.enter_context(tc.tile_pool(name="xp", bufs=3))
    hp = ctx.enter_context(tc.tile_pool(name="hp", bufs=4))
    scp = ctx.enter_context(tc.tile_pool(name="scp", bufs=2)) if EVENSCAN else None
    gp = ctx.enter_context(tc.tile_pool(name="gp", bufs=3))
    pp = ctx.enter_context(tc.tile_pool(name="pp", bufs=2, space="PSUM"))
    rp = ctx.enter_context(tc.tile_pool(name="rp", bufs=3))
    obp = ctx.enter_context(tc.tile_pool(name="obp", bufs=3))

    # PE clock warm-up: ~4.5us of back-to-back matmuls into a scratch PSUM
    # tile (recycled by the pool for the first real chunk) so the HAM clock
    # gate reaches 8/8 before the first chunk's matmuls issue.
    warm = pp.tile([128, W], F32, tag="front")
    for _ in range(80):
        nc.tensor.matmul(
            warm[:, 0:128], wt["wb31_blk"][:], wt["wb31_blk"][:],
            start=True, stop=True,
        )

    scan31_eng = nc.gpsimd if SCAN31_ON_GPS else nc.vector

    def one_pass():
      for img in range(n_img):
        hs: dict[int, object] = {}

        def produce(t):
            xt = xp.tile([128, SCANW], F16, tag="xt")
            nc.sync.dma_start(xt[:], x_d[img, 128 * t : 128 * (t + 1), :])
            ht = hp.tile([128, HW_], F16, tag="ht")
            if EVENSCAN:
                # even-lag scans (4B-aligned fp16 -> 2x eligible):
                #   s12[t] = sum(xs[t+1 : t+13]),  s32[t] = sum(xs[t+1 : t+33])
                # then h11[w] = s12[w+26] - xs[w+38]  (11-window at w)
                #      h31[w] = s32[w+16] - xs[w+48]  (31-window at w)
                sc = scp.tile([128, HW_], F16, tag="sc")
                nc.vector.tensor_tensor_scan(
                    sc[:, 0:L11], xt[:, 12 : 12 + L11], xt[:, 0:L11], 0.0, ADD, SUB
                )
                nc.vector.tensor_tensor_scan(
                    sc[:, HOFF31 : HOFF31 + L31],
                    xt[:, 32 : 32 + L31],
                    xt[:, 0:L31],
                    0.0,
                    ADD,
                    SUB,
                )
                nc.vector.tensor_sub(
                    ht[:, O11 : O11 + W], sc[:, 26 : 26 + W], xt[:, 38 : 38 + W]
                )
                nc.vector.tensor_sub(
                    ht[:, HOFF31 + O31 : HOFF31 + O31 + W],
                    sc[:, HOFF31 + 16 : HOFF31 + 16 + W],
                    xt[:, 48 : 48 + W],
                )
            else:
                # h11: state = (xs[t+11] + state) - xs[t]
                nc.vector.tensor_tensor_scan(
                    ht[:, 0:L11], xt[:, 11 : 11 + L11], xt[:, 0:L11], 0.0, ADD, SUB
                )
                # h31: state = (xs[t+31] + state) - xs[t]
                scan31_eng.tensor_tensor_scan(
                    ht[:, HOFF31 : HOFF31 + L31],
                    xt[:, 31 : 31 + L31],
                    xt[:, 0:L31],
                    0.0,
                    ADD,
                    SUB,
                )
            hs[t] = ht

        def consume(t):
            # halo slab: inner chunks rows 0..14 = prev ht[113:128],
            # 15..29 = next ht[0:15]; edge chunks use a 15-row slab.
            g = gp.tile([2 * R_OUT, HW_], F16, tag="g")
            if t == 0:
                nc.sync.dma_start(g[0:R_OUT, :], hs[t + 1][0:R_OUT, :])
                sfx, gsl = "N", slice(0, R_OUT)
            elif t == CHUNKS - 1:
                nc.sync.dma_start(g[0:R_OUT, :], hs[t - 1][128 - R_OUT : 128, :])
                sfx, gsl = "P", slice(0, R_OUT)
            else:
                nc.sync.dma_start(g[0:R_OUT, :], hs[t - 1][128 - R_OUT : 128, :])
                nc.sync.dma_start(g[R_OUT : 2 * R_OUT, :], hs[t + 1][0:R_OUT, :])
                sfx, gsl = "", slice(0, 2 * R_OUT)

            ht = hs[t]
            psf = pp.tile([128, W], F32, tag="front")
            psb = pp.tile([128, W], F32, tag="back")
            MM = nc.tensor.matmul
            # grouped by weight so consecutive matmuls share one LDWEIGHTS;
            # with the fused ratio op, psb only accumulates the 31-box (the
            # -c*front term is folded into the custom DVE op)
            plan = [
                ("wf_blk", psf, "h11", True, False),
                ("wf_halo" + sfx, psf, "g11", False, True),
                ("wb31_blk", psb, "h31", True, False),
                ("wb31_halo" + sfx, psb, "g31", False, True),
            ]
            if not FUSED_RATIO:
                plan[-1] = ("wb31_halo" + sfx, psb, "g31", False, False)
                plan += [
                    ("wn11_blk", psb, "h11", False, False),
                    ("wn11_halo" + sfx, psb, "g11", False, True),
                ]
            for wname, dst, rhs_of, st, sp in plan:
                for h0 in (0, 512):
                    s = slice(h0, h0 + 512)
                    c11 = slice(O11 + h0, O11 + h0 + 512)
                    c31 = slice(HOFF31 + O31 + h0, HOFF31 + O31 + h0 + 512)
                    rhs = {
                        "h11": ht[:, c11],
                        "h31": ht[:, c31],
                        "g11": g[gsl, c11],
                        "g31": g[gsl, c31],
                    }[rhs_of]
                    MM(dst[:, s], wt[wname][:], rhs, start=st, stop=sp)

            o1 = obp.tile([128, W], F16, tag="o1")
            nc.scalar.copy(o1[:], psf[:])
            o0 = obp.tile([128, W], F16, tag="o0")
            if FUSED_RATIO:
                nc.vector._custom_dve(
                    RATIO_OP, out=o0[:], in0=psb[:], in1=o1[:],
                    s0=RECIP_C0, s1=RECIP_C1, imm2=NRM_C,
                )
            else:
                r = rp.tile([128, W], F32, tag="r")
                nc.vector.reciprocal_approx_fast(out=r[:], in_=psb[:])
                nc.vector.tensor_mul(o0[:], psf[:], r[:])
            rows = slice(128 * t, 128 * (t + 1))
            nc.scalar.dma_start(o_d[img, rows, :], o0[:])
            nc.scalar.dma_start(o_d[n_img + img, rows, :], o1[:])

        produce(0)
        produce(1)
        consume(0)
        for t in range(2, CHUNKS):
            produce(t)
            consume(t - 1)
        consume(CHUNKS - 1)

    if reps == 1:
        one_pass()
    else:
        with tc.For_i(0, reps, 1):
            one_pass()


def build(n_img: int = BPC, reps: int = 1):
    nc = bacc.Bacc("TRN2", target_bir_lowering=False, debug=False)
    x_d = nc.dram_tensor("x", [n_img, H, SCANW], F16, kind="ExternalInput").ap()
    o_d = nc.dram_tensor("out", [2 * n_img, H, W], F16, kind="ExternalOutput").ap()
    wts = _weights()
    w_d = {
        k: nc.dram_tensor(k, list(v.shape), F16, kind="ExternalInput").ap()
        for k, v in wts.items()
    }
    with tile.TileContext(nc) as tc:
        _cfar_tile_kernel(tc, x_d, o_d, w_d, n_img, reps)
    nc.compile()
    return nc, wts


_CACHE: dict = {}


def make_in_maps(x: np.ndarray, wts: dict) -> list[dict]:
    xs = np.zeros((B, H, SCANW), dtype=np.float16)
    xs[:, :, PADL : PADL + W] = x.reshape(B, H, W)
    in_maps = []
    for i in range(NCORES):
        m = {"x": np.ascontiguousarray(xs[BPC * i : BPC * (i + 1)])}
        m.update(wts)
        in_maps.append(m)
    return in_maps


def kernel(x: np.ndarray) -> np.ndarray:
    x = np.ascontiguousarray(np.asarray(x, dtype=np.float32))
    assert x.shape == (B, 1, H, W), x.shape
    if "nc" not in _CACHE:
        _CACHE["nc"], _CACHE["wts"] = build(BPC)
    nc, wts = _CACHE["nc"], _CACHE["wts"]
    in_maps = make_in_maps(x, wts)
    res = run_bass_kernel_spmd(nc, in_maps, list(range(NCORES))).results
    out = np.empty((2 * B, 1, H, W), dtype=np.float32)
    for i in range(NCORES):
        o = res[i]["out"].astype(np.float32)
        out[BPC * i : BPC * (i + 1), 0] = o[:BPC]
        out[B + BPC * i : B + BPC * (i + 1), 0] = o[BPC:]
    return out
